# revision 36
# baseline (speedup 1.0000x reference)
"""Multi-head causal attention (B=2, S=2048, H=1024, 16 heads x 64, RoPE) on 8 trn2 cores.

Sharding: tensor-parallel over heads (2 heads/core) for QKV+attention, then
AllToAlls switch to token-parallel for the output projection. Each core owns
4x128-token slices (one per batch-half); the host concatenates row slices.

Key structure (per core c, heads h0=2c, h0+1):
 - xT [1024, 4096] feature-major bf16 activations (host-transposed), one DMA
   per 512-token chunk. QKV = 24 matmuls/chunk from a [128, 8, 384] W tile.
 - RoPE feature-major in bf16 (2x DVE modes) with host cos/sin tables; the
   half-rotation partition swap is 4 SBUF->SBUF DMAs from a scratch tile
   (no WAR hazard), sin-mul on GpSimd, cos-mul + add on DVE.
 - Scores TRANSPOSED in bf16: sT[k, q] = matmul(lhsT=kT_blk, rhs=qT_chunk),
   both heads in one [128, 2, 512] PSUM tile -> ONE merged exp per k-block.
   Softmax max-subtraction skipped (logits ~N(0,1)). Causal mask = bf16 0/1
   multiply on the diagonal block.
 - PV FLIPPED: ctx[q, d] = matmul(lhsT=probsT[k, q-blk], rhs=[V | ones]).
   Cost follows the 65-wide free dim, and the ones column accumulates the
   softmax sums for free (column 64).
 - Normalize: sums are a per-partition scalar -> DVE reciprocal + GpSimd
   tensor_scalar multiplies; ctx -> ctxT via PE transpose + DVE copy.
 - FOUR collectives (one per batch-half, [8, 128, 128] bf16) so only the
   last 256KB AllToAll is exposed in the tail; out-projections for earlier
   halves run as soon as their collective lands.
 - Emission uses a filler pump: phase-1 QKV m-groups / V-transposes /
   out-projections are interleaved between attention k-block units so the
   PE never idles (and stays p-state ramped) while ACT streams exps.
"""

from collections import deque

import numpy as np

import concourse.bacc as bacc
import concourse.mybir as mybir
import concourse.tile as tile
from concourse.bass_utils import run_bass_kernel_spmd

F32 = mybir.dt.float32
BF16 = mybir.dt.bfloat16
EXP = mybir.ActivationFunctionType.Exp

B, S, H = 2, 2048, 1024
NH, HD = 16, 64
NCORES = 8
T = B * S            # 4096 flattened tokens (b-major)
TBLK = T // NCORES   # 512 tokens per core
P = 128


def _build_nc():
    nc = bacc.Bacc(None, num_devices=NCORES)

    xT_d = nc.dram_tensor("xT", [H, T], BF16, kind="ExternalInput")
    wqkvT_d = nc.dram_tensor("wqkvT", [H, 384], BF16, kind="ExternalInput")
    woutT_d = nc.dram_tensor("woutT", [H, H], BF16, kind="ExternalInput")
    costab_d = nc.dram_tensor("costab", [P, S], BF16, kind="ExternalInput")
    sintab_d = nc.dram_tensor("sintab", [P, S], BF16, kind="ExternalInput")
    maskT_d = nc.dram_tensor("maskT", [P, 2 * P], BF16, kind="ExternalInput")
    identf_d = nc.dram_tensor("identf", [P, P], BF16, kind="ExternalInput")
    out_d = nc.dram_tensor("out", [TBLK, H], F32, kind="ExternalOutput")

    with tile.TileContext(nc) as tc:
        with (
            tc.tile_pool(name="long", bufs=1) as lp,
            tc.tile_pool(name="dram", bufs=1, space="DRAM") as dp,
            tc.tile_pool(name="p1s", bufs=4) as p1s,
            tc.tile_pool(name="p1v", bufs=2) as p1v,
            tc.tile_pool(name="p1t", bufs=3) as p1t,
            tc.tile_pool(name="ps1", bufs=1, space="PSUM") as ps1,
            tc.tile_pool(name="ps1t", bufs=1, space="PSUM") as ps1t,
            tc.tile_pool(name="p2", bufs=8) as p2,
            tc.tile_pool(name="p2n", bufs=3) as p2n,
            tc.tile_pool(name="ps2s", bufs=2, space="PSUM") as ps2s,
            tc.tile_pool(name="ps2c", bufs=1, space="PSUM") as ps2c,
            tc.tile_pool(name="p3", bufs=2) as p3,
        ):
            # long-lived tiles
            qT = [lp.tile([P, S], BF16, tag=f"qT{b}", name=f"qT{b}") for b in range(B)]
            kT = [lp.tile([P, S], BF16, tag=f"kT{b}", name=f"kT{b}") for b in range(B)]
            V = [[lp.tile([P, 16, 65], BF16, tag=f"V{b}{h}", name=f"V{b}{h}")
                  for h in range(2)] for b in range(B)]
            ctxT = [lp.tile([P, S], BF16, tag=f"ctxT{b}", name=f"ctxT{b}")
                    for b in range(B)]
            maskT_t = lp.tile([P, 2, P], BF16, tag="maskT")
            identf_t = lp.tile([P, P], BF16, tag="identf")
            wo = lp.tile([P, 8, H], BF16, tag="wo")
            wq = lp.tile([P, 8, 384], BF16, tag="wq")
            costab_t = lp.tile([P, S], BF16, tag="costab")
            sintab_t = lp.tile([P, S], BF16, tag="sintab")

            nc.sync.dma_start(wq[:], wqkvT_d[:].rearrange("(k p) c -> p k c", p=P))

            def load_xt(b, ch):
                tok0 = b * S + ch * 512
                xt = p1s.tile([P, 8, 512], BF16, tag="xt", name=f"xt{b}{ch}")
                nc.sync.dma_start(
                    xt[:], xT_d[:, tok0:tok0 + 512]
                    .rearrange("(k p) t -> p k t", p=P))
                return xt

            xt00 = load_xt(0, 0)
            nc.sync.dma_start(
                maskT_t[:], maskT_d[:].rearrange("p (h k) -> p h k", h=2))
            nc.sync.dma_start(identf_t[:], identf_d[:])
            nc.sync.dma_start(costab_t[:], costab_d[:])
            nc.sync.dma_start(sintab_t[:], sintab_d[:])
            for b in range(B):
                for h in range(2):
                    nc.vector.memset(V[b][h][:, :, 64:65], 1.0)

            a2a_in = [[dp.tile([NCORES, P, P], BF16, name=f"a2a_in{b}{f}",
                               tag=f"a2a_in{b}{f}") for f in range(2)]
                      for b in range(B)]
            a2a_out = [[dp.tile([NCORES, P, P], BF16, name=f"a2a_out{b}{f}",
                                tag=f"a2a_out{b}{f}") for f in range(2)]
                       for b in range(B)]

            def qkv_m(b, ch, m, xt, st):
                """One QKV output-tile: 8 matmuls + RoPE or V staging.
                q and k stage into one [P, 2, 512] tmp so the partition-swap
                is 4 chunk-level DMAs instead of 8."""
                c0 = ch * 512
                ps = ps1.tile([P, 512], F32, tag="qkv_ps")
                for kt in range(8):
                    nc.tensor.matmul(
                        ps[:], wq[:, kt, m * P:(m + 1) * P], xt[:, kt, :],
                        start=(kt == 0), stop=(kt == 7),
                    )
                if m < 2:
                    if m == 0:
                        st["tmp"] = p1t.tile([P, 2, 512], BF16, tag="tmp", name="tmp")
                        st["swp"] = p1t.tile([P, 2, 512], BF16, tag="swp", name="swp")
                    nc.vector.tensor_copy(st["tmp"][:, m, :], ps[:])
                    if m == 1:
                        tmp, swp = st["tmp"], st["swp"]
                        for g in range(4):
                            dst = g * 32
                            srcp = dst ^ 32
                            nc.sync.dma_start(swp[dst:dst + 32],
                                              tmp[srcp:srcp + 32])
                        for mm, tgt in ((0, qT[b]), (1, kT[b])):
                            nc.gpsimd.tensor_mul(
                                swp[:, mm, :], swp[:, mm, :],
                                sintab_t[:, c0:c0 + 512])
                            nc.vector.tensor_mul(
                                tgt[:, c0:c0 + 512], tmp[:, mm, :],
                                costab_t[:, c0:c0 + 512])
                            nc.vector.tensor_add(
                                tgt[:, c0:c0 + 512], tgt[:, c0:c0 + 512],
                                swp[:, mm, :])
                    return None
                vt = p1v.tile([P, 512], BF16, tag="vT", name=f"vT{b}{ch}")
                nc.vector.tensor_copy(vt[:], ps[:])
                return vt

            def v_transposes(b, ch, vt):
                for h in range(2):
                    pst = ps1t.tile([P, 256], BF16, tag="tp", name="vt_ps")
                    for tb in range(4):
                        nc.tensor.transpose(
                            pst[:, tb * HD:(tb + 1) * HD],
                            vt[h * HD:(h + 1) * HD, tb * P:(tb + 1) * P],
                            identf_t[h * HD:(h + 1) * HD, h * HD:(h + 1) * HD],
                        )
                    nc.vector.tensor_copy(
                        V[b][h][:, ch * 4:(ch + 1) * 4, 0:HD],
                        pst[:].rearrange("p (g d) -> p g d", g=4))

            xts = {}

            def p1_units(b, ch):
                """Filler units for one 512-token chunk of QKV+RoPE+V
                (the xt load for (b, ch) must be emitted beforehand)."""
                state = {"vt": None, "st": {}}

                def u_m(m):
                    r = qkv_m(b, ch, m, xts[(b, ch)], state["st"])
                    if r is not None:
                        state["vt"] = r

                def u_vt():
                    v_transposes(b, ch, state["vt"])

                units = [lambda m=m: u_m(m) for m in range(3)]
                units.append(u_vt)
                return units

            fillers = deque()

            def pump():
                if fillers:
                    fillers.popleft()()

            def drain(n=None):
                cnt = len(fillers) if n is None else n
                for _ in range(cnt):
                    pump()

            def _pv_group(pctx, b, qs, qb, pbs_all):
                """All of query-block qb's PV accumulation as one contiguous
                PSUM group (banks allow only one open group at a time)."""
                last = 4 * qs + qb
                for h in range(2):
                    for kb in range(last + 1):
                        nc.tensor.matmul(
                            pctx[:, 2 * qb + h, 0:65],
                            pbs_all[kb][:, h, qb * P:(qb + 1) * P],
                            V[b][h][:, kb, :],
                            start=(kb == 0), stop=(kb == last),
                            skip_group_check=True,
                        )

            def p2_qs(b, qs):
                """Attention for one 512-query chunk: scores, exp, flipped PV,
                normalize, transpose back to ctxT. Pumps one filler unit per
                k-block to keep the PE busy while ACT runs the exps."""
                nkb = 4 * qs + 4
                pctx = ps2c.tile([P, 8, P], F32, tag="ctx", name="pctx")
                rb = p2n.tile([P, 4, 2, 1], F32, tag="recip")

                def normalize(qb):
                    # region qb of pctx just received its last accumulation
                    nc.vector.reciprocal(
                        rb[:, qb, :, :], pctx[:, 2 * qb:2 * qb + 2, 64:65])
                    cs = p2n.tile([P, 2, HD], BF16, tag="csb", bufs=4)
                    for h in range(2):
                        nc.vector.tensor_scalar_mul(
                            cs[:, h, :], pctx[:, 2 * qb + h, 0:HD],
                            rb[:, qb, h, 0:1])
                    pt = ps1t.tile([P, 256], BF16, tag="tp", name="ctxt_ps")
                    nc.tensor.transpose(pt[:, 0:P], cs[:], identf_t[:])
                    q0 = qs * 512 + qb * P
                    nc.vector.tensor_copy(ctxT[b][:, q0:q0 + P], pt[:, 0:P])

                pbs_all = []
                for kb in range(nkb):
                    j = kb - 4 * qs
                    qoff = max(0, j) * P
                    psT = ps2s.tile([P, 2, 512], F32, tag="sT")
                    for h in range(2):
                        nc.tensor.matmul(
                            psT[:, h, qoff:512],
                            kT[b][h * HD:(h + 1) * HD, kb * P:(kb + 1) * P],
                            qT[b][h * HD:(h + 1) * HD,
                                  qs * 512 + qoff:(qs + 1) * 512],
                            start=True, stop=True,
                            tile_position=(h * HD, 0),
                            skip_group_check=True,
                        )
                    pb = p2.tile([P, 2, 512], BF16, tag="probs", bufs=18)
                    nc.scalar.activation(
                        pb[:, :, qoff:512], psT[:, :, qoff:512], EXP,
                        scale=0.125)
                    if j >= 0:
                        nc.vector.tensor_mul(
                            pb[:, :, qoff:qoff + P],
                            pb[:, :, qoff:qoff + P], maskT_t[:])
                    pbs_all.append(pb)
                    pump()
                    if j >= 0:
                        # query-block j's last k-block just got its probs:
                        # emit its full PV accumulation + normalize
                        _pv_group(pctx, b, qs, j, pbs_all)
                        normalize(j)
                if qs % 2 == 1:
                    half = qs // 2
                    nc.sync.dma_start(
                        a2a_in[b][half][:].rearrange("g p t -> p g t"),
                        ctxT[b][:, half * 1024:(half + 1) * 1024]
                        .rearrange("p (g t) -> p g t", g=8))
                    nc.gpsimd.collective_compute(
                        "AllToAll",
                        mybir.AluOpType.bypass,
                        replica_groups=[list(range(NCORES))],
                        ins=[a2a_in[b][half].opt()],
                        outs=[a2a_out[b][half].opt()],
                    )

            ctxs_t = {}

            def ctxs_load(bb, half):
                ctxs = p3.tile([P, 8, P], BF16, tag="ctxs",
                               name=f"ctxs{bb}{half}")
                nc.sync.dma_start(
                    ctxs[:], a2a_out[bb][half][:].rearrange("j p t -> p j t"))
                ctxs_t[(bb, half)] = ctxs

            def out_proj(bb, half):
                """Out-projection for this core's 128 tokens of one
                batch-half; lands in out_d rows [bb*256+half*128, +128)."""
                ctxs = ctxs_t[(bb, half)]
                r0 = bb * 256 + half * P
                for nt in range(2):
                    po = ps1.tile([P, 512], F32, tag="qkv_ps", name="po")
                    for jj in range(8):
                        nc.tensor.matmul(
                            po[:],
                            ctxs[:, jj, :],
                            wo[:, jj, nt * 512:(nt + 1) * 512],
                            start=(jj == 0), stop=(jj == 7),
                        )
                    ob = p3.tile([P, 512], F32, tag="ob", name="ob", bufs=3)
                    nc.vector.tensor_copy(ob[:], po[:])
                    nc.sync.dma_start(
                        out_d[r0:r0 + P, nt * 512:(nt + 1) * 512], ob[:])

            # ---- emission schedule ----
            st00 = {}
            xts[(0, 0)] = xt00
            qkv_m(0, 0, 0, xt00, st00)
            xts[(0, 1)] = load_xt(0, 1)
            qkv_m(0, 0, 1, xt00, st00)
            vt00 = qkv_m(0, 0, 2, xt00, st00)
            nc.sync.dma_start(
                wo[:], woutT_d[:].rearrange("(j p) n -> p j n", p=P))

            def u_load(b, ch):
                return lambda: xts.__setitem__((b, ch), load_xt(b, ch))

            # each chunk's xt load is pumped ~4 units (one chunk) ahead
            chunks = [(0, 1), (0, 2), (0, 3), (1, 0), (1, 1), (1, 2), (1, 3)]
            fillers.append(lambda: v_transposes(0, 0, vt00))
            for i, (b, ch) in enumerate(chunks):
                if i + 1 < len(chunks):
                    fillers.append(u_load(*chunks[i + 1]))
                fillers.extend(p1_units(b, ch))
            n_units = len(fillers)  # 35

            for qs in range(4):
                p2_qs(0, qs)
                # chunk qs+1 of batch 0 must be fully emitted before its
                # attention chunk (scores need qT/kT, PV needs V)
                if qs < 3:
                    while n_units - len(fillers) < 1 + 5 * (qs + 1):
                        pump()
                if qs == 1:
                    fillers.append(lambda: ctxs_load(0, 0))
            drain()  # finish all of batch 1's QKV before its attention
            # out-proj units are appended only once their collective is
            # close to done: an early-pumped unit head-of-line blocks the
            # in-order PE queue on the a2a wait
            fillers.append(lambda: out_proj(0, 0))
            for qs in range(4):
                p2_qs(1, qs)
                if qs == 0:
                    fillers.append(lambda: ctxs_load(0, 1))
                if qs == 1:
                    fillers.append(lambda: out_proj(0, 1))
                    fillers.append(lambda: ctxs_load(1, 0))
                if qs == 2:
                    fillers.append(lambda: out_proj(1, 0))
            drain()
            ctxs_load(1, 1)
            out_proj(1, 1)

    nc.finalize()
    return nc


_NC_CACHE = None


def _get_nc():
    global _NC_CACHE
    if _NC_CACHE is None:
        _NC_CACHE = _build_nc()
    return _NC_CACHE


def _host_tables():
    j = np.arange(32)
    inv = (10000.0 ** (-(j.astype(np.float64)) / 32.0))
    pos = np.arange(S, dtype=np.float64)
    fr = pos[:, None] * inv[None, :]              # [S, 32]
    import ml_dtypes
    cosT = np.cos(fr).T.astype(np.float32)        # [32, S]
    sinT = np.sin(fr).T.astype(np.float32)
    costab = np.tile(cosT, (4, 1)).astype(ml_dtypes.bfloat16)
    sintab = np.concatenate([-sinT, sinT, -sinT, sinT], 0).astype(
        ml_dtypes.bfloat16)
    kk = np.arange(P)[:, None]
    qq = np.arange(P)[None, :]
    mask1 = np.where(kk <= qq, 1.0, 0.0)
    maskT = np.concatenate([mask1, mask1], axis=1).astype(ml_dtypes.bfloat16)
    identf = np.eye(P, dtype=np.float32).astype(ml_dtypes.bfloat16)
    return costab, sintab, maskT, identf


def _make_in_maps(x, W_qkv, W_out):
    import ml_dtypes
    costab, sintab, maskT, identf = _host_tables()
    xT = np.ascontiguousarray(x.reshape(T, H).T).astype(ml_dtypes.bfloat16)
    woutT = np.ascontiguousarray(W_out.T).astype(ml_dtypes.bfloat16)
    in_maps = []
    for c in range(NCORES):
        h0 = 2 * c
        rows = np.concatenate([
            W_qkv[HD * h0:HD * (h0 + 2)],
            W_qkv[H + HD * h0:H + HD * (h0 + 2)],
            W_qkv[2 * H + HD * h0:2 * H + HD * (h0 + 2)],
        ], axis=0)                                        # [384, H]
        wqkvT = np.ascontiguousarray(rows.T).astype(ml_dtypes.bfloat16)
        in_maps.append({
            "xT": xT, "wqkvT": wqkvT, "woutT": woutT,
            "costab": costab, "sintab": sintab,
            "maskT": maskT, "identf": identf,
        })
    return in_maps


def _run_spmd(x, W_qkv, W_out, **kw):
    nc = _get_nc()
    in_maps = _make_in_maps(x, W_qkv, W_out)
    return run_bass_kernel_spmd(nc, in_maps, core_ids=list(range(NCORES)),
                                **kw)


def kernel(x, W_qkv, W_out):
    x = np.asarray(x, dtype=np.float32)
    W_qkv = np.asarray(W_qkv, dtype=np.float32)
    W_out = np.asarray(W_out, dtype=np.float32)
    res = _run_spmd(x, W_qkv, W_out)
    # core c owns, per batch b and half f, tokens [f*1024 + c*128, +128)
    full = np.empty((T, H), dtype=np.float32)
    for c in range(NCORES):
        o = res.results[c]["out"]
        for b in range(B):
            for f in range(2):
                t0 = b * S + f * 1024 + c * P
                full[t0:t0 + P] = o[b * 256 + f * P:b * 256 + (f + 1) * P]
    return full.reshape(B, S, H)


# revision 41
# speedup vs baseline: 1.0007x; 1.0007x over previous
"""Multi-head causal attention (B=2, S=2048, H=1024, 16 heads x 64, RoPE) on 8 trn2 cores.

Sharding: tensor-parallel over heads (2 heads/core) for QKV+attention, then
AllToAlls switch to token-parallel for the output projection. Each core owns
4x128-token slices (one per batch-half); the host concatenates row slices.

Key structure (per core c, heads h0=2c, h0+1):
 - xT [1024, 4096] feature-major bf16 activations (host-transposed), one DMA
   per 512-token chunk. QKV = 24 matmuls/chunk from a [128, 8, 384] W tile.
 - RoPE feature-major in bf16 (2x DVE modes) with host cos/sin tables; the
   half-rotation partition swap is 4 SBUF->SBUF DMAs from a scratch tile
   (no WAR hazard), sin-mul on GpSimd, cos-mul + add on DVE.
 - Scores TRANSPOSED in bf16: sT[k, q] = matmul(lhsT=kT_blk, rhs=qT_chunk),
   both heads in one [128, 2, 512] PSUM tile -> ONE merged exp per k-block.
   Softmax max-subtraction skipped (logits ~N(0,1)). Causal mask = bf16 0/1
   multiply on the diagonal block.
 - PV FLIPPED: ctx[q, d] = matmul(lhsT=probsT[k, q-blk], rhs=[V | ones]).
   Cost follows the 65-wide free dim, and the ones column accumulates the
   softmax sums for free (column 64).
 - Normalize: sums are a per-partition scalar -> DVE reciprocal +
   tensor_scalar multiplies; ctx -> ctxT via PE transpose + DVE copy.
   PV accumulation runs qb-major: each query-block's k-accumulation is one
   contiguous PSUM group (banks allow only one open group at a time).
 - FOUR collectives (one per batch-half, [8, 128, 128] bf16) so only the
   last 256KB AllToAll is exposed in the tail; out-projections for earlier
   halves run as soon as their collective lands.
 - Emission uses a filler pump: phase-1 QKV m-groups / V-transposes /
   out-projections are interleaved between attention k-block units so the
   PE never idles (and stays p-state ramped) while ACT streams exps.
"""

from collections import deque

import numpy as np

import concourse.bacc as bacc
import concourse.mybir as mybir
import concourse.tile as tile
from concourse.bass_utils import run_bass_kernel_spmd

F32 = mybir.dt.float32
BF16 = mybir.dt.bfloat16
EXP = mybir.ActivationFunctionType.Exp

B, S, H = 2, 2048, 1024
NH, HD = 16, 64
NCORES = 8
T = B * S            # 4096 flattened tokens (b-major)
TBLK = T // NCORES   # 512 tokens per core
P = 128


def _build_nc():
    nc = bacc.Bacc(None, num_devices=NCORES)

    xT_d = nc.dram_tensor("xT", [H, T], BF16, kind="ExternalInput")
    wqkvT_d = nc.dram_tensor("wqkvT", [H, 384], BF16, kind="ExternalInput")
    woutT_d = nc.dram_tensor("woutT", [H, H], BF16, kind="ExternalInput")
    costab_d = nc.dram_tensor("costab", [P, S], BF16, kind="ExternalInput")
    sintab_d = nc.dram_tensor("sintab", [P, S], BF16, kind="ExternalInput")
    maskT_d = nc.dram_tensor("maskT", [P, 2 * P], BF16, kind="ExternalInput")
    identf_d = nc.dram_tensor("identf", [P, P], BF16, kind="ExternalInput")
    out_d = nc.dram_tensor("out", [TBLK, H], F32, kind="ExternalOutput")

    with tile.TileContext(nc) as tc:
        with (
            tc.tile_pool(name="long", bufs=1) as lp,
            tc.tile_pool(name="dram", bufs=1, space="DRAM") as dp,
            tc.tile_pool(name="p1s", bufs=4) as p1s,
            tc.tile_pool(name="p1v", bufs=2) as p1v,
            tc.tile_pool(name="p1t", bufs=3) as p1t,
            tc.tile_pool(name="ps1", bufs=1, space="PSUM") as ps1,
            tc.tile_pool(name="ps1t", bufs=1, space="PSUM") as ps1t,
            tc.tile_pool(name="p2", bufs=8) as p2,
            tc.tile_pool(name="p2n", bufs=3) as p2n,
            tc.tile_pool(name="ps2s", bufs=2, space="PSUM") as ps2s,
            tc.tile_pool(name="ps2c", bufs=1, space="PSUM") as ps2c,
            tc.tile_pool(name="p3", bufs=2) as p3,
        ):
            # long-lived tiles
            qT = [lp.tile([P, S], BF16, tag=f"qT{b}", name=f"qT{b}") for b in range(B)]
            kT = [lp.tile([P, S], BF16, tag=f"kT{b}", name=f"kT{b}") for b in range(B)]
            V = [[lp.tile([P, 16, 65], BF16, tag=f"V{b}{h}", name=f"V{b}{h}")
                  for h in range(2)] for b in range(B)]
            ctxT = [lp.tile([P, S], BF16, tag=f"ctxT{b}", name=f"ctxT{b}")
                    for b in range(B)]
            maskT_t = lp.tile([P, 2, P], BF16, tag="maskT")
            identf_t = lp.tile([P, P], BF16, tag="identf")
            wo = lp.tile([P, 8, H], BF16, tag="wo")
            wq = lp.tile([P, 8, 384], BF16, tag="wq")
            costab_t = lp.tile([P, S], BF16, tag="costab")
            sintab_t = lp.tile([P, S], BF16, tag="sintab")

            nc.sync.dma_start(wq[:], wqkvT_d[:].rearrange("(k p) c -> p k c", p=P))

            def load_xt(b, ch):
                tok0 = b * S + ch * 512
                xt = p1s.tile([P, 8, 512], BF16, tag="xt", name=f"xt{b}{ch}")
                nc.sync.dma_start(
                    xt[:], xT_d[:, tok0:tok0 + 512]
                    .rearrange("(k p) t -> p k t", p=P))
                return xt

            xt00 = load_xt(0, 0)
            nc.sync.dma_start(
                maskT_t[:], maskT_d[:].rearrange("p (h k) -> p h k", h=2))
            nc.sync.dma_start(identf_t[:], identf_d[:])
            nc.sync.dma_start(costab_t[:], costab_d[:])
            nc.sync.dma_start(sintab_t[:], sintab_d[:])
            for b in range(B):
                for h in range(2):
                    nc.vector.memset(V[b][h][:, :, 64:65], 1.0)

            a2a_in = [[dp.tile([NCORES, P, P], BF16, name=f"a2a_in{b}{f}",
                               tag=f"a2a_in{b}{f}") for f in range(2)]
                      for b in range(B)]
            a2a_out = [[dp.tile([NCORES, P, P], BF16, name=f"a2a_out{b}{f}",
                                tag=f"a2a_out{b}{f}") for f in range(2)]
                       for b in range(B)]

            def qkv_m(b, ch, m, xt, st):
                """One QKV output-tile: 8 matmuls + RoPE or V staging.
                q and k stage into one [P, 2, 512] tmp so the partition-swap
                is 4 chunk-level DMAs instead of 8."""
                c0 = ch * 512
                ps = ps1.tile([P, 512], F32, tag="qkv_ps")
                for kt in range(8):
                    nc.tensor.matmul(
                        ps[:], wq[:, kt, m * P:(m + 1) * P], xt[:, kt, :],
                        start=(kt == 0), stop=(kt == 7),
                    )
                if m < 2:
                    if m == 0:
                        st["tmp"] = p1t.tile([P, 2, 512], BF16, tag="tmp", name="tmp")
                        st["swp"] = p1t.tile([P, 2, 512], BF16, tag="swp", name="swp")
                    nc.vector.tensor_copy(st["tmp"][:, m, :], ps[:])
                    if m == 1:
                        tmp, swp = st["tmp"], st["swp"]
                        for g in range(4):
                            dst = g * 32
                            srcp = dst ^ 32
                            nc.sync.dma_start(swp[dst:dst + 32],
                                              tmp[srcp:srcp + 32])
                        for mm, tgt in ((0, qT[b]), (1, kT[b])):
                            nc.gpsimd.tensor_mul(
                                swp[:, mm, :], swp[:, mm, :],
                                sintab_t[:, c0:c0 + 512])
                            nc.vector.tensor_mul(
                                tgt[:, c0:c0 + 512], tmp[:, mm, :],
                                costab_t[:, c0:c0 + 512])
                            nc.vector.tensor_add(
                                tgt[:, c0:c0 + 512], tgt[:, c0:c0 + 512],
                                swp[:, mm, :])
                    return None
                vt = p1v.tile([P, 512], BF16, tag="vT", name=f"vT{b}{ch}")
                nc.vector.tensor_copy(vt[:], ps[:])
                return vt

            def v_transposes(b, ch, vt):
                for h in range(2):
                    pst = ps1t.tile([P, 256], BF16, tag="tp", name="vt_ps")
                    for tb in range(4):
                        nc.tensor.transpose(
                            pst[:, tb * HD:(tb + 1) * HD],
                            vt[h * HD:(h + 1) * HD, tb * P:(tb + 1) * P],
                            identf_t[h * HD:(h + 1) * HD, h * HD:(h + 1) * HD],
                        )
                    nc.vector.tensor_copy(
                        V[b][h][:, ch * 4:(ch + 1) * 4, 0:HD],
                        pst[:].rearrange("p (g d) -> p g d", g=4))

            xts = {}

            def p1_units(b, ch):
                """Filler units for one 512-token chunk of QKV+RoPE+V
                (the xt load for (b, ch) must be emitted beforehand)."""
                state = {"vt": None, "st": {}}

                def u_m(m):
                    r = qkv_m(b, ch, m, xts[(b, ch)], state["st"])
                    if r is not None:
                        state["vt"] = r

                def u_vt():
                    v_transposes(b, ch, state["vt"])

                units = [lambda m=m: u_m(m) for m in range(3)]
                units.append(u_vt)
                return units

            fillers = deque()

            def pump():
                if fillers:
                    fillers.popleft()()

            def drain(n=None):
                cnt = len(fillers) if n is None else n
                for _ in range(cnt):
                    pump()

            def _pv_group(pctx, b, qs, qb, pbs_all):
                """All of query-block qb's PV accumulation as one contiguous
                PSUM group (banks allow only one open group at a time)."""
                last = 4 * qs + qb
                for h in range(2):
                    for kb in range(last + 1):
                        nc.tensor.matmul(
                            pctx[:, 2 * qb + h, 0:65],
                            pbs_all[kb][:, h, qb * P:(qb + 1) * P],
                            V[b][h][:, kb, :],
                            start=(kb == 0), stop=(kb == last),
                            skip_group_check=True,
                        )

            def p2_qs(b, qs):
                """Attention for one 512-query chunk: scores, exp, flipped PV,
                normalize, transpose back to ctxT. Pumps one filler unit per
                k-block to keep the PE busy while ACT runs the exps."""
                nkb = 4 * qs + 4
                pctx = ps2c.tile([P, 8, P], F32, tag="ctx", name="pctx")
                rb = p2n.tile([P, 4, 2, 1], F32, tag="recip")

                def normalize(qb):
                    # region qb of pctx just received its last accumulation
                    nc.vector.reciprocal(
                        rb[:, qb, :, :], pctx[:, 2 * qb:2 * qb + 2, 64:65])
                    cs = p2n.tile([P, 2, HD], BF16, tag="csb", bufs=4)
                    for h in range(2):
                        nc.vector.tensor_scalar_mul(
                            cs[:, h, :], pctx[:, 2 * qb + h, 0:HD],
                            rb[:, qb, h, 0:1])
                    pt = ps1t.tile([P, 256], BF16, tag="tp", name="ctxt_ps")
                    nc.tensor.transpose(pt[:, 0:P], cs[:], identf_t[:])
                    q0 = qs * 512 + qb * P
                    nc.vector.tensor_copy(ctxT[b][:, q0:q0 + P], pt[:, 0:P])

                pbs_all = []
                for kb in range(nkb):
                    j = kb - 4 * qs
                    qoff = max(0, j) * P
                    psT = ps2s.tile([P, 2, 512], F32, tag="sT")
                    for h in range(2):
                        nc.tensor.matmul(
                            psT[:, h, qoff:512],
                            kT[b][h * HD:(h + 1) * HD, kb * P:(kb + 1) * P],
                            qT[b][h * HD:(h + 1) * HD,
                                  qs * 512 + qoff:(qs + 1) * 512],
                            start=True, stop=True,
                            tile_position=(h * HD, 0),
                            skip_group_check=True,
                        )
                    pb = p2.tile([P, 2, 512], BF16, tag="probs", bufs=26)
                    nc.scalar.activation(
                        pb[:, :, qoff:512], psT[:, :, qoff:512], EXP,
                        scale=0.125)
                    if j >= 0:
                        nc.vector.tensor_mul(
                            pb[:, :, qoff:qoff + P],
                            pb[:, :, qoff:qoff + P], maskT_t[:])
                    pbs_all.append(pb)
                    pump()
                    if j >= 0:
                        # query-block j's last k-block just got its probs:
                        # emit its full PV accumulation + normalize
                        _pv_group(pctx, b, qs, j, pbs_all)
                        normalize(j)
                if qs % 2 == 1:
                    half = qs // 2
                    nc.sync.dma_start(
                        a2a_in[b][half][:].rearrange("g p t -> p g t"),
                        ctxT[b][:, half * 1024:(half + 1) * 1024]
                        .rearrange("p (g t) -> p g t", g=8))
                    nc.gpsimd.collective_compute(
                        "AllToAll",
                        mybir.AluOpType.bypass,
                        replica_groups=[list(range(NCORES))],
                        ins=[a2a_in[b][half].opt()],
                        outs=[a2a_out[b][half].opt()],
                    )

            ctxs_t = {}

            def ctxs_load(bb, half):
                ctxs = p3.tile([P, 8, P], BF16, tag="ctxs",
                               name=f"ctxs{bb}{half}")
                nc.sync.dma_start(
                    ctxs[:], a2a_out[bb][half][:].rearrange("j p t -> p j t"))
                ctxs_t[(bb, half)] = ctxs

            def out_proj(bb, half):
                """Out-projection for this core's 128 tokens of one
                batch-half; lands in out_d rows [bb*256+half*128, +128)."""
                ctxs = ctxs_t[(bb, half)]
                r0 = bb * 256 + half * P
                for nt in range(2):
                    po = ps1.tile([P, 512], F32, tag="qkv_ps", name="po")
                    for jj in range(8):
                        nc.tensor.matmul(
                            po[:],
                            ctxs[:, jj, :],
                            wo[:, jj, nt * 512:(nt + 1) * 512],
                            start=(jj == 0), stop=(jj == 7),
                        )
                    ob = p3.tile([P, 512], F32, tag="ob", name="ob", bufs=3)
                    nc.vector.tensor_copy(ob[:], po[:])
                    nc.sync.dma_start(
                        out_d[r0:r0 + P, nt * 512:(nt + 1) * 512], ob[:])

            # ---- emission schedule ----
            st00 = {}
            xts[(0, 0)] = xt00
            qkv_m(0, 0, 0, xt00, st00)
            xts[(0, 1)] = load_xt(0, 1)
            qkv_m(0, 0, 1, xt00, st00)
            vt00 = qkv_m(0, 0, 2, xt00, st00)
            nc.sync.dma_start(
                wo[:], woutT_d[:].rearrange("(j p) n -> p j n", p=P))

            def u_load(b, ch):
                return lambda: xts.__setitem__((b, ch), load_xt(b, ch))

            # each chunk's xt load is pumped ~4 units (one chunk) ahead
            chunks = [(0, 1), (0, 2), (0, 3), (1, 0), (1, 1), (1, 2), (1, 3)]
            fillers.append(lambda: v_transposes(0, 0, vt00))
            for i, (b, ch) in enumerate(chunks):
                if i + 1 < len(chunks):
                    fillers.append(u_load(*chunks[i + 1]))
                fillers.extend(p1_units(b, ch))
            n_units = len(fillers)  # 35

            for qs in range(4):
                p2_qs(0, qs)
                # chunk qs+1 of batch 0 must be fully emitted before its
                # attention chunk (scores need qT/kT, PV needs V)
                if qs < 3:
                    while n_units - len(fillers) < 1 + 5 * (qs + 1):
                        pump()
                if qs == 1:
                    fillers.append(lambda: ctxs_load(0, 0))
            drain()  # finish all of batch 1's QKV before its attention
            # out-proj units are appended only once their collective is
            # close to done: an early-pumped unit head-of-line blocks the
            # in-order PE queue on the a2a wait
            fillers.append(lambda: out_proj(0, 0))
            for qs in range(4):
                p2_qs(1, qs)
                if qs == 0:
                    fillers.append(lambda: ctxs_load(0, 1))
                if qs == 1:
                    fillers.append(lambda: out_proj(0, 1))
                    fillers.append(lambda: ctxs_load(1, 0))
                if qs == 2:
                    fillers.append(lambda: out_proj(1, 0))
            drain()
            ctxs_load(1, 1)
            out_proj(1, 1)

    nc.finalize()
    return nc


_NC_CACHE = None


def _get_nc():
    global _NC_CACHE
    if _NC_CACHE is None:
        _NC_CACHE = _build_nc()
    return _NC_CACHE


def _host_tables():
    j = np.arange(32)
    inv = (10000.0 ** (-(j.astype(np.float64)) / 32.0))
    pos = np.arange(S, dtype=np.float64)
    fr = pos[:, None] * inv[None, :]              # [S, 32]
    import ml_dtypes
    cosT = np.cos(fr).T.astype(np.float32)        # [32, S]
    sinT = np.sin(fr).T.astype(np.float32)
    costab = np.tile(cosT, (4, 1)).astype(ml_dtypes.bfloat16)
    sintab = np.concatenate([-sinT, sinT, -sinT, sinT], 0).astype(
        ml_dtypes.bfloat16)
    kk = np.arange(P)[:, None]
    qq = np.arange(P)[None, :]
    mask1 = np.where(kk <= qq, 1.0, 0.0)
    maskT = np.concatenate([mask1, mask1], axis=1).astype(ml_dtypes.bfloat16)
    identf = np.eye(P, dtype=np.float32).astype(ml_dtypes.bfloat16)
    return costab, sintab, maskT, identf


def _make_in_maps(x, W_qkv, W_out):
    import ml_dtypes
    costab, sintab, maskT, identf = _host_tables()
    xT = np.ascontiguousarray(x.reshape(T, H).T).astype(ml_dtypes.bfloat16)
    woutT = np.ascontiguousarray(W_out.T).astype(ml_dtypes.bfloat16)
    in_maps = []
    for c in range(NCORES):
        h0 = 2 * c
        rows = np.concatenate([
            W_qkv[HD * h0:HD * (h0 + 2)],
            W_qkv[H + HD * h0:H + HD * (h0 + 2)],
            W_qkv[2 * H + HD * h0:2 * H + HD * (h0 + 2)],
        ], axis=0)                                        # [384, H]
        wqkvT = np.ascontiguousarray(rows.T).astype(ml_dtypes.bfloat16)
        in_maps.append({
            "xT": xT, "wqkvT": wqkvT, "woutT": woutT,
            "costab": costab, "sintab": sintab,
            "maskT": maskT, "identf": identf,
        })
    return in_maps


def _run_spmd(x, W_qkv, W_out, **kw):
    nc = _get_nc()
    in_maps = _make_in_maps(x, W_qkv, W_out)
    return run_bass_kernel_spmd(nc, in_maps, core_ids=list(range(NCORES)),
                                **kw)


def kernel(x, W_qkv, W_out):
    x = np.asarray(x, dtype=np.float32)
    W_qkv = np.asarray(W_qkv, dtype=np.float32)
    W_out = np.asarray(W_out, dtype=np.float32)
    res = _run_spmd(x, W_qkv, W_out)
    # core c owns, per batch b and half f, tokens [f*1024 + c*128, +128)
    full = np.empty((T, H), dtype=np.float32)
    for c in range(NCORES):
        o = res.results[c]["out"]
        for b in range(B):
            for f in range(2):
                t0 = b * S + f * 1024 + c * P
                full[t0:t0 + P] = o[b * 256 + f * P:b * 256 + (f + 1) * P]
    return full.reshape(B, S, H)


# revision 44
# speedup vs baseline: 1.0023x; 1.0016x over previous
"""Multi-head causal attention (B=2, S=2048, H=1024, 16 heads x 64, RoPE) on 8 trn2 cores.

Sharding: tensor-parallel over heads (2 heads/core) for QKV+attention, then
AllToAlls switch to token-parallel for the output projection. Each core owns
4x128-token slices (one per batch-half); the host concatenates row slices.

Key structure (per core c, heads h0=2c, h0+1):
 - xT [1024, 4096] feature-major bf16 activations (host-transposed), one DMA
   per 512-token chunk. QKV = 24 matmuls/chunk from a [128, 8, 384] W tile.
 - RoPE feature-major in bf16 (2x DVE modes) with host cos/sin tables; the
   half-rotation partition swap is 4 SBUF->SBUF DMAs from a scratch tile
   (no WAR hazard), sin-mul on GpSimd, cos-mul + add on DVE.
 - Scores TRANSPOSED in bf16: sT[k, q] = matmul(lhsT=kT_blk, rhs=qT_chunk),
   both heads in one [128, 2, 512] PSUM tile -> ONE merged exp per k-block.
   Softmax max-subtraction skipped (logits ~N(0,1)). Causal mask = bf16 0/1
   multiply on the diagonal block.
 - PV FLIPPED: ctx[q, d] = matmul(lhsT=probsT[k, q-blk], rhs=[V | ones]).
   Cost follows the 65-wide free dim, and the ones column accumulates the
   softmax sums for free (column 64).
 - Normalize: sums are a per-partition scalar -> DVE reciprocal +
   tensor_scalar multiplies; ctx -> ctxT via PE transpose + DVE copy.
   PV accumulation runs qb-major: each query-block's k-accumulation is one
   contiguous PSUM group (banks allow only one open group at a time).
 - FOUR collectives (one per batch-half, [8, 128, 128] bf16) so only the
   last 256KB AllToAll is exposed in the tail; out-projections for earlier
   halves run as soon as their collective lands.
 - Emission uses a filler pump: phase-1 QKV m-groups / V-transposes /
   out-projections are interleaved between attention k-block units so the
   PE never idles (and stays p-state ramped) while ACT streams exps.
"""

from collections import deque

import numpy as np

import concourse.bacc as bacc
import concourse.mybir as mybir
import concourse.tile as tile
from concourse.bass_utils import run_bass_kernel_spmd

F32 = mybir.dt.float32
BF16 = mybir.dt.bfloat16
EXP = mybir.ActivationFunctionType.Exp

B, S, H = 2, 2048, 1024
NH, HD = 16, 64
NCORES = 8
T = B * S            # 4096 flattened tokens (b-major)
TBLK = T // NCORES   # 512 tokens per core
P = 128


def _build_nc():
    nc = bacc.Bacc(None, num_devices=NCORES)

    xT_d = nc.dram_tensor("xT", [H, T], BF16, kind="ExternalInput")
    wqkvT_d = nc.dram_tensor("wqkvT", [H, 384], BF16, kind="ExternalInput")
    woutT_d = nc.dram_tensor("woutT", [H, H], BF16, kind="ExternalInput")
    costab_d = nc.dram_tensor("costab", [P, S], BF16, kind="ExternalInput")
    sintab_d = nc.dram_tensor("sintab", [P, S], BF16, kind="ExternalInput")
    maskT_d = nc.dram_tensor("maskT", [P, 2 * P], BF16, kind="ExternalInput")
    identf_d = nc.dram_tensor("identf", [P, P], BF16, kind="ExternalInput")
    out_d = nc.dram_tensor("out", [TBLK, H], F32, kind="ExternalOutput")

    with tile.TileContext(nc) as tc:
        with (
            tc.tile_pool(name="long", bufs=1) as lp,
            tc.tile_pool(name="dram", bufs=1, space="DRAM") as dp,
            tc.tile_pool(name="p1s", bufs=4) as p1s,
            tc.tile_pool(name="p1v", bufs=2) as p1v,
            tc.tile_pool(name="p1t", bufs=3) as p1t,
            tc.tile_pool(name="ps1", bufs=1, space="PSUM") as ps1,
            tc.tile_pool(name="ps1t", bufs=1, space="PSUM") as ps1t,
            tc.tile_pool(name="p2", bufs=8) as p2,
            tc.tile_pool(name="p2n", bufs=3) as p2n,
            tc.tile_pool(name="ps2s", bufs=2, space="PSUM") as ps2s,
            tc.tile_pool(name="ps2c", bufs=1, space="PSUM") as ps2c,
            tc.tile_pool(name="p3", bufs=2) as p3,
        ):
            # long-lived tiles
            qT = [lp.tile([P, S], BF16, tag=f"qT{b}", name=f"qT{b}") for b in range(B)]
            kT = [lp.tile([P, S], BF16, tag=f"kT{b}", name=f"kT{b}") for b in range(B)]
            V = [[lp.tile([P, 16, 65], BF16, tag=f"V{b}{h}", name=f"V{b}{h}")
                  for h in range(2)] for b in range(B)]
            ctxT = [lp.tile([P, S], BF16, tag=f"ctxT{b}", name=f"ctxT{b}")
                    for b in range(B)]
            maskT_t = lp.tile([P, 2, P], BF16, tag="maskT")
            identf_t = lp.tile([P, P], BF16, tag="identf")
            wo = lp.tile([P, 8, H], BF16, tag="wo")
            wq = lp.tile([P, 8, 384], BF16, tag="wq")
            costab_t = lp.tile([P, S], BF16, tag="costab")
            sintab_t = lp.tile([P, S], BF16, tag="sintab")

            nc.sync.dma_start(wq[:], wqkvT_d[:].rearrange("(k p) c -> p k c", p=P))

            def load_xt(b, ch):
                tok0 = b * S + ch * 512
                xt = p1s.tile([P, 8, 512], BF16, tag="xt", name=f"xt{b}{ch}")
                nc.sync.dma_start(
                    xt[:], xT_d[:, tok0:tok0 + 512]
                    .rearrange("(k p) t -> p k t", p=P))
                return xt

            xt00 = load_xt(0, 0)
            nc.sync.dma_start(
                maskT_t[:], maskT_d[:].rearrange("p (h k) -> p h k", h=2))
            nc.sync.dma_start(identf_t[:], identf_d[:])
            nc.sync.dma_start(costab_t[:], costab_d[:])
            nc.sync.dma_start(sintab_t[:], sintab_d[:])
            for b in range(B):
                for h in range(2):
                    nc.vector.memset(V[b][h][:, :, 64:65], 1.0)

            a2a_in = [[dp.tile([NCORES, P, P], BF16, name=f"a2a_in{b}{f}",
                               tag=f"a2a_in{b}{f}") for f in range(2)]
                      for b in range(B)]
            a2a_out = [[dp.tile([NCORES, P, P], BF16, name=f"a2a_out{b}{f}",
                                tag=f"a2a_out{b}{f}") for f in range(2)]
                       for b in range(B)]

            def qkv_m(b, ch, m, xt, st):
                """One QKV output-tile: 8 matmuls + RoPE or V staging.
                q and k stage into one [P, 2, 512] tmp so the partition-swap
                is 4 chunk-level DMAs instead of 8."""
                c0 = ch * 512
                ps = ps1.tile([P, 512], F32, tag="qkv_ps")
                for kt in range(8):
                    nc.tensor.matmul(
                        ps[:], wq[:, kt, m * P:(m + 1) * P], xt[:, kt, :],
                        start=(kt == 0), stop=(kt == 7),
                    )
                if m < 2:
                    if m == 0:
                        st["tmp"] = p1t.tile([P, 2, 512], BF16, tag="tmp", name="tmp")
                        st["swp"] = p1t.tile([P, 2, 512], BF16, tag="swp", name="swp")
                    nc.vector.tensor_copy(st["tmp"][:, m, :], ps[:])
                    if m == 1:
                        tmp, swp = st["tmp"], st["swp"]
                        for g in range(4):
                            dst = g * 32
                            srcp = dst ^ 32
                            nc.sync.dma_start(swp[dst:dst + 32],
                                              tmp[srcp:srcp + 32])
                        for mm, tgt in ((0, qT[b]), (1, kT[b])):
                            nc.gpsimd.tensor_mul(
                                swp[:, mm, :], swp[:, mm, :],
                                sintab_t[:, c0:c0 + 512])
                            nc.vector.tensor_mul(
                                tgt[:, c0:c0 + 512], tmp[:, mm, :],
                                costab_t[:, c0:c0 + 512])
                            nc.vector.tensor_add(
                                tgt[:, c0:c0 + 512], tgt[:, c0:c0 + 512],
                                swp[:, mm, :])
                    return None
                vt = p1v.tile([P, 512], BF16, tag="vT", name=f"vT{b}{ch}")
                nc.vector.tensor_copy(vt[:], ps[:])
                return vt

            def v_transposes(b, ch, vt):
                for h in range(2):
                    pst = ps1t.tile([P, 256], BF16, tag="tp", name="vt_ps")
                    for tb in range(4):
                        nc.tensor.transpose(
                            pst[:, tb * HD:(tb + 1) * HD],
                            vt[h * HD:(h + 1) * HD, tb * P:(tb + 1) * P],
                            identf_t[h * HD:(h + 1) * HD, h * HD:(h + 1) * HD],
                        )
                    nc.vector.tensor_copy(
                        V[b][h][:, ch * 4:(ch + 1) * 4, 0:HD],
                        pst[:].rearrange("p (g d) -> p g d", g=4))

            xts = {}

            def p1_units(b, ch):
                """Filler units for one 512-token chunk of QKV+RoPE+V
                (the xt load for (b, ch) must be emitted beforehand)."""
                state = {"vt": None, "st": {}}

                def u_m(m):
                    r = qkv_m(b, ch, m, xts[(b, ch)], state["st"])
                    if r is not None:
                        state["vt"] = r

                def u_vt():
                    v_transposes(b, ch, state["vt"])

                units = [lambda m=m: u_m(m) for m in range(3)]
                units.append(u_vt)
                return units

            fillers = deque()

            def pump():
                if fillers:
                    fillers.popleft()()

            def drain(n=None):
                cnt = len(fillers) if n is None else n
                for _ in range(cnt):
                    pump()

            def _pv_group(pctx, b, qs, qb, pbs_all):
                """All of query-block qb's PV accumulation as one contiguous
                PSUM group (banks allow only one open group at a time)."""
                last = 4 * qs + qb
                for h in range(2):
                    for kb in range(last + 1):
                        nc.tensor.matmul(
                            pctx[:, 2 * qb + h, 0:65],
                            pbs_all[kb][:, h, qb * P:(qb + 1) * P],
                            V[b][h][:, kb, :],
                            start=(kb == 0), stop=(kb == last),
                            skip_group_check=True,
                        )

            def p2_qs(b, qs):
                """Attention for one 512-query chunk: scores, exp, flipped PV,
                normalize, transpose back to ctxT. Pumps one filler unit per
                k-block to keep the PE busy while ACT runs the exps."""
                nkb = 4 * qs + 4
                pctx = ps2c.tile([P, 8, P], F32, tag="ctx", name="pctx")
                rb = p2n.tile([P, 4, 2, 1], F32, tag="recip")

                def normalize(qb):
                    # region qb of pctx just received its last accumulation
                    nc.vector.reciprocal(
                        rb[:, qb, :, :], pctx[:, 2 * qb:2 * qb + 2, 64:65])
                    cs = p2n.tile([P, 2, HD], BF16, tag="csb", bufs=4)
                    for h in range(2):
                        nc.vector.tensor_scalar_mul(
                            cs[:, h, :], pctx[:, 2 * qb + h, 0:HD],
                            rb[:, qb, h, 0:1])
                    pt = ps1t.tile([P, 256], BF16, tag="tp", name="ctxt_ps")
                    nc.tensor.transpose(pt[:, 0:P], cs[:], identf_t[:])
                    q0 = qs * 512 + qb * P
                    nc.vector.tensor_copy(ctxT[b][:, q0:q0 + P], pt[:, 0:P])

                pbs_all = []
                for kb in range(nkb):
                    j = kb - 4 * qs
                    qoff = max(0, j) * P
                    psT = ps2s.tile([P, 2, 512], F32, tag="sT")
                    for h in range(2):
                        nc.tensor.matmul(
                            psT[:, h, qoff:512],
                            kT[b][h * HD:(h + 1) * HD, kb * P:(kb + 1) * P],
                            qT[b][h * HD:(h + 1) * HD,
                                  qs * 512 + qoff:(qs + 1) * 512],
                            start=True, stop=True,
                            tile_position=(h * HD, 0),
                            skip_group_check=True,
                        )
                    pb = p2.tile([P, 2, 512], BF16, tag="probs", bufs=26)
                    nc.scalar.activation(
                        pb[:, :, qoff:512], psT[:, :, qoff:512], EXP,
                        scale=0.125)
                    if j >= 0:
                        nc.vector.tensor_mul(
                            pb[:, :, qoff:qoff + P],
                            pb[:, :, qoff:qoff + P], maskT_t[:])
                    pbs_all.append(pb)
                    pump()
                    if j >= 0:
                        # query-block j's last k-block just got its probs:
                        # emit its full PV accumulation + normalize
                        _pv_group(pctx, b, qs, j, pbs_all)
                        normalize(j)
                if qs % 2 == 1:
                    half = qs // 2
                    nc.sync.dma_start(
                        a2a_in[b][half][:].rearrange("g p t -> p g t"),
                        ctxT[b][:, half * 1024:(half + 1) * 1024]
                        .rearrange("p (g t) -> p g t", g=8))
                    nc.gpsimd.collective_compute(
                        "AllToAll",
                        mybir.AluOpType.bypass,
                        replica_groups=[list(range(NCORES))],
                        ins=[a2a_in[b][half].opt()],
                        outs=[a2a_out[b][half].opt()],
                    )

            ctxs_t = {}

            def ctxs_load(bb, half):
                ctxs = p3.tile([P, 8, P], BF16, tag="ctxs",
                               name=f"ctxs{bb}{half}")
                nc.sync.dma_start(
                    ctxs[:], a2a_out[bb][half][:].rearrange("j p t -> p j t"))
                ctxs_t[(bb, half)] = ctxs

            def out_proj(bb, half):
                """Out-projection for this core's 128 tokens of one
                batch-half; lands in out_d rows [bb*256+half*128, +128)."""
                ctxs = ctxs_t[(bb, half)]
                r0 = bb * 256 + half * P
                for nt in range(2):
                    po = ps1.tile([P, 512], F32, tag="qkv_ps", name="po")
                    for jj in range(8):
                        nc.tensor.matmul(
                            po[:],
                            ctxs[:, jj, :],
                            wo[:, jj, nt * 512:(nt + 1) * 512],
                            start=(jj == 0), stop=(jj == 7),
                        )
                    if bb == 1 and half == 1:
                        # exposed tail: split copy+store so the second half's
                        # DMA overlaps the first half's copy
                        for q in range(2):
                            c0, c1 = nt * 512 + q * 256, nt * 512 + (q + 1) * 256
                            obq = p3.tile([P, 256], F32, tag="obq",
                                          name="obq", bufs=4)
                            nc.vector.tensor_copy(obq[:], po[:, q * 256:(q + 1) * 256])
                            nc.sync.dma_start(out_d[r0:r0 + P, c0:c1], obq[:])
                    else:
                        ob = p3.tile([P, 512], F32, tag="ob", name="ob",
                                     bufs=3)
                        nc.vector.tensor_copy(ob[:], po[:])
                        nc.sync.dma_start(
                            out_d[r0:r0 + P, nt * 512:(nt + 1) * 512], ob[:])

            # ---- emission schedule ----
            st00 = {}
            xts[(0, 0)] = xt00
            qkv_m(0, 0, 0, xt00, st00)
            xts[(0, 1)] = load_xt(0, 1)
            qkv_m(0, 0, 1, xt00, st00)
            vt00 = qkv_m(0, 0, 2, xt00, st00)
            nc.sync.dma_start(
                wo[:], woutT_d[:].rearrange("(j p) n -> p j n", p=P))

            def u_load(b, ch):
                return lambda: xts.__setitem__((b, ch), load_xt(b, ch))

            # each chunk's xt load is pumped ~4 units (one chunk) ahead
            chunks = [(0, 1), (0, 2), (0, 3), (1, 0), (1, 1), (1, 2), (1, 3)]
            fillers.append(lambda: v_transposes(0, 0, vt00))
            for i, (b, ch) in enumerate(chunks):
                if i + 1 < len(chunks):
                    fillers.append(u_load(*chunks[i + 1]))
                fillers.extend(p1_units(b, ch))
            n_units = len(fillers)  # 35

            for qs in range(4):
                p2_qs(0, qs)
                # chunk qs+1 of batch 0 must be fully emitted before its
                # attention chunk (scores need qT/kT, PV needs V)
                if qs < 3:
                    while n_units - len(fillers) < 1 + 5 * (qs + 1):
                        pump()
                if qs == 1:
                    fillers.append(lambda: ctxs_load(0, 0))
            drain()  # finish all of batch 1's QKV before its attention
            # out-proj units are appended only once their collective is
            # close to done: an early-pumped unit head-of-line blocks the
            # in-order PE queue on the a2a wait
            fillers.append(lambda: out_proj(0, 0))
            for qs in range(4):
                p2_qs(1, qs)
                if qs == 0:
                    fillers.append(lambda: ctxs_load(0, 1))
                if qs == 1:
                    fillers.append(lambda: out_proj(0, 1))
                    fillers.append(lambda: ctxs_load(1, 0))
                if qs == 2:
                    fillers.append(lambda: out_proj(1, 0))
            drain()
            ctxs_load(1, 1)
            out_proj(1, 1)

    nc.finalize()
    return nc


_NC_CACHE = None


def _get_nc():
    global _NC_CACHE
    if _NC_CACHE is None:
        _NC_CACHE = _build_nc()
    return _NC_CACHE


def _host_tables():
    j = np.arange(32)
    inv = (10000.0 ** (-(j.astype(np.float64)) / 32.0))
    pos = np.arange(S, dtype=np.float64)
    fr = pos[:, None] * inv[None, :]              # [S, 32]
    import ml_dtypes
    cosT = np.cos(fr).T.astype(np.float32)        # [32, S]
    sinT = np.sin(fr).T.astype(np.float32)
    costab = np.tile(cosT, (4, 1)).astype(ml_dtypes.bfloat16)
    sintab = np.concatenate([-sinT, sinT, -sinT, sinT], 0).astype(
        ml_dtypes.bfloat16)
    kk = np.arange(P)[:, None]
    qq = np.arange(P)[None, :]
    mask1 = np.where(kk <= qq, 1.0, 0.0)
    maskT = np.concatenate([mask1, mask1], axis=1).astype(ml_dtypes.bfloat16)
    identf = np.eye(P, dtype=np.float32).astype(ml_dtypes.bfloat16)
    return costab, sintab, maskT, identf


def _make_in_maps(x, W_qkv, W_out):
    import ml_dtypes
    costab, sintab, maskT, identf = _host_tables()
    xT = np.ascontiguousarray(x.reshape(T, H).T).astype(ml_dtypes.bfloat16)
    woutT = np.ascontiguousarray(W_out.T).astype(ml_dtypes.bfloat16)
    in_maps = []
    for c in range(NCORES):
        h0 = 2 * c
        rows = np.concatenate([
            W_qkv[HD * h0:HD * (h0 + 2)],
            W_qkv[H + HD * h0:H + HD * (h0 + 2)],
            W_qkv[2 * H + HD * h0:2 * H + HD * (h0 + 2)],
        ], axis=0)                                        # [384, H]
        wqkvT = np.ascontiguousarray(rows.T).astype(ml_dtypes.bfloat16)
        in_maps.append({
            "xT": xT, "wqkvT": wqkvT, "woutT": woutT,
            "costab": costab, "sintab": sintab,
            "maskT": maskT, "identf": identf,
        })
    return in_maps


def _run_spmd(x, W_qkv, W_out, **kw):
    nc = _get_nc()
    in_maps = _make_in_maps(x, W_qkv, W_out)
    return run_bass_kernel_spmd(nc, in_maps, core_ids=list(range(NCORES)),
                                **kw)


def kernel(x, W_qkv, W_out):
    x = np.asarray(x, dtype=np.float32)
    W_qkv = np.asarray(W_qkv, dtype=np.float32)
    W_out = np.asarray(W_out, dtype=np.float32)
    res = _run_spmd(x, W_qkv, W_out)
    # core c owns, per batch b and half f, tokens [f*1024 + c*128, +128)
    full = np.empty((T, H), dtype=np.float32)
    for c in range(NCORES):
        o = res.results[c]["out"]
        for b in range(B):
            for f in range(2):
                t0 = b * S + f * 1024 + c * P
                full[t0:t0 + P] = o[b * 256 + f * P:b * 256 + (f + 1) * P]
    return full.reshape(B, S, H)
